# revision 21
# baseline (speedup 1.0000x reference)
"""AttnBlock (GroupNorm -> 1x1 qkv conv -> full HW x HW attention -> 1x1 proj
-> residual) on 8 Trainium2 NeuronCores.

Sharding: 8 cores = 4 batch elements x 2 query-halves. Each core receives its
batch element's full x[b] (pixel axis rolled so the core's query half sits in
columns 0..2047), computes GroupNorm + full K/V + Q for its half, runs
attention over key blocks, and the output projection. The host transposes the
1x1-conv weights, adds proj bias + residual, and gathers.

Raw Bass (explicit per-engine streams + semaphores; this toolchain's walrus
rejects the multi-wait instructions Tile emits). Compute dtype bf16 for all
big matmuls (fp32 accumulation in PSUM); GroupNorm statistics in fp32.

Device layouts (partition dim first):
  h  = groupnormed x, bf16   [C=512 -> 4 tiles of 128, HW=4096]
  Q  = wqT.T @ h (+bq)       [512 -> 4 tiles, 2048]
  K  = wkT.T @ h (+bk)       [512 -> 4 tiles, 4096]
  Vt = h.T @ wvT (+bv)       [128, 32 j-blocks, 512]   (pixels on partitions)
  scores_t = K.T @ Q         [128 keys, 512 queries] psum, per (j, i-quarter)
  probs    = exp(scores * C^-0.5), bf16   (no max subtraction; |scores| < ~6)
  O       += Vt_j.T @ probs_j   [4 x [128, 512]] psum accumulated over j
  sums    += ones.T @ probs_j   [1, 512] psum
  out = (wpT.T @ O) * (1/sums broadcast)  -> DRAM [512, 2048] f32
"""

from contextlib import ExitStack

import numpy as np

import concourse.bass as bass
from concourse import mybir
from concourse.bass_utils import run_bass_kernel_spmd

F32 = mybir.dt.float32
BF16 = mybir.dt.bfloat16

B, C, H, W = 4, 512, 64, 64
HW = H * W              # 4096 pixels
NG = 32                 # groupnorm groups
GS = C // NG            # 16 channels per group
P = 128                 # SBUF partitions
KC = C // P             # 4 channel chunks
NQ = HW // 2            # 2048 queries per core
F = 512                 # free-dim tile (one PSUM bank of f32)
NJ = HW // P            # 32 key blocks
NQF = NQ // F           # 4 query quarters
NGT = P // GS           # 8 groups per channel tile
EPS = 1e-6
SCALE = float(C) ** -0.5
AF = mybir.ActivationFunctionType
ALU = mybir.AluOpType


def build_nc() -> bass.Bass:
    nc = bass.Bass()

    x_d = nc.dram_tensor("x", [C, HW], F32, kind="ExternalInput")
    w_d = {}
    for nm in ("wqT", "wkT", "wvT", "wpT"):
        w_d[nm] = nc.dram_tensor(nm, [C, C], F32, kind="ExternalInput")
    bq_d = nc.dram_tensor("bq", [C, 1], F32, kind="ExternalInput")
    bk_d = nc.dram_tensor("bk", [C, 1], F32, kind="ExternalInput")
    bvb_d = nc.dram_tensor("bvb", [P, C], F32, kind="ExternalInput")
    gsc_d = nc.dram_tensor("gscale", [C, 1], F32, kind="ExternalInput")
    gbi_d = nc.dram_tensor("gbias", [C, 1], F32, kind="ExternalInput")
    gmat_d = nc.dram_tensor("gmat", [P, NGT], F32, kind="ExternalInput")
    gexp_d = nc.dram_tensor("gexp", [NGT, P], F32, kind="ExternalInput")
    out_d = nc.dram_tensor("out", [C, NQ], F32, kind="ExternalOutput")

    ctx = ExitStack()
    with ctx:
        # ---------------- SBUF ----------------
        def sb(shape, dt, name):
            return ctx.enter_context(nc.sbuf_tensor(name, shape, dt))
        x_sb = [sb([P, HW], F32, f"x{k}") for k in range(2)]        # 32KB/p
        h_sb = [sb([P, HW], BF16, f"h{k}") for k in range(KC)]      # 32KB/p
        q_sb = [sb([P, NQ], BF16, f"q{k}") for k in range(KC)]      # 16KB/p
        k_sb = [sb([P, HW], BF16, f"kk{k}") for k in range(KC)]     # 32KB/p
        vt_sb = sb([P, NJ, F], BF16, "vt")                          # 32KB/p
        wstage = [sb([P, C], F32, f"wstage{i}") for i in range(2)]  # 4KB/p
        w_sb = {nm: [sb([P, C], BF16, f"{nm}{k}") for k in range(KC)]
                for nm in ("wqT", "wkT", "wvT", "wpT")}             # 16KB/p
        bvb_sb = sb([P, C], F32, "bvb_sb")
        gmat_sb = sb([P, NGT], F32, "gmat_sb")
        gexp_sb = sb([NGT, P], F32, "gexp_sb")
        bq_sb = [sb([P, 1], F32, f"bq_sb{k}") for k in range(KC)]
        bk_sb = [sb([P, 1], F32, f"bk_sb{k}") for k in range(KC)]
        gsc_sb = [sb([P, 1], F32, f"gsc_sb{k}") for k in range(KC)]
        gbi_sb = [sb([P, 1], F32, f"gbi_sb{k}") for k in range(KC)]
        ones_col = sb([P, 1], BF16, "ones_col")
        ones_row = sb([1, P], F32, "ones_row")
        zero_col = sb([P, 1], F32, "zero_col")
        # groupnorm scratch (per c-tile, reused)
        stats = sb([P, HW // F, 6], F32, "stats")
        mv = sb([P, 2], F32, "mv")
        st2 = sb([P, 2], F32, "st2")
        g2 = sb([NGT, 2], F32, "g2")
        gv = sb([NGT, 1], F32, "gv")
        eps_sb = sb([NGT, 1], F32, "eps_sb")
        chs = sb([P, 2], F32, "chs")
        av = sb([P, 1], F32, "av")
        bv_ = sb([P, 1], F32, "bv_")
        # attention scratch
        probs = [sb([P, F], BF16, f"probs{i}") for i in range(2)]
        recip = sb([1, F], F32, "recip")
        rb_sb = sb([P, F], F32, "rb_sb")
        o_sb = [sb([P, F], BF16, f"o_sb{i}") for i in range(KC)]
        ot = [sb([P, F], F32, f"ot{i}") for i in range(2)]

        # ---------------- PSUM (8 banks) ----------------
        def ps(shape, name):
            return ctx.enter_context(nc.psum_tensor(name, shape, F32))
        s_ps = [ps([P, F], f"s_ps{i}") for i in range(2)]
        o_ps = [ps([P, F], f"o_ps{i}") for i in range(KC)]
        sums_ps = ps([1, F], "sums_ps")
        aux_ps = ps([P, F], "aux_ps")   # gn pg/pb + recip broadcast

        # ---------------- semaphores ----------------
        def sem(name):
            return ctx.enter_context(nc.semaphore(name))
        dma_x = [sem("dma_x0"), sem("dma_x1")]   # +16 per x tile, by slot
        dma_w = [sem("dma_w0"), sem("dma_w1")]   # +16 per wstage load
        dma_m = sem("dma_m")        # +16 per misc const load
        dma_o = [sem("dma_o0"), sem("dma_o1")]   # +16 per output store
        s_wcvt = sem("s_wcvt")      # DVE memsets (4) + weight converts (16)
        s_dve = sem("s_dve")        # serialized gn DVE chain (21 per c-tile)
        s_rb = sem("s_rb")          # rb_sb copy per quarter (DVE)
        s_gn_pe = sem("s_gn_pe")    # gn PE matmuls (2 per c-tile)
        s_gn_act = sem("s_gn_act")  # gn sqrt (1 per c-tile)
        s_h = sem("s_h")            # normalized h tiles
        s_qg_pe = sem("s_qg_pe")    # qkv matmul groups done (PE)
        s_qg_dve = sem("s_qg_dve")  # qkv drains done (DVE)
        s_sc = sem("s_sc")          # scores groups (PE)
        s_exp = sem("s_exp")        # exps (ACT)
        s_att = sem("s_att")        # attnV+sums groups (PE)
        s_recip = sem("s_recip")    # recip per quarter (DVE)
        s_bcast = sem("s_bcast")    # bcast matmul per quarter (PE)
        s_osb = sem("s_osb")        # o_sb drains (DVE)
        s_pp = sem("s_pp")          # proj matmul groups (PE)
        s_ot = sem("s_ot")          # ot muls (DVE)

        NMISC = 3 + 4 * KC          # gmat, gexp, bvb, per-k consts
        W_ORDER = ("wqT", "wkT", "wvT", "wpT")

        # qkv "groups" in PE emission order
        qkv_groups = ([("v", j) for j in range(NJ)]
                      + [("q", m, n) for m in range(KC)
                         for n in range(NQ // F)]
                      + [("k", m, n) for m in range(KC)
                         for n in range(HW // F)])
        NQG = len(qkv_groups)

        with nc.Block() as block:

            # ================= GPSIMD: all DMA =================
            @block.gpsimd
            def _(g: bass.BassEngine):
                g.dma_start(out=gmat_sb[:, :], in_=gmat_d[:, :]).then_inc(
                    dma_m, 16)
                g.dma_start(out=gexp_sb[:, :], in_=gexp_d[:, :]).then_inc(
                    dma_m, 16)
                g.dma_start(out=bvb_sb[:, :], in_=bvb_d[:, :]).then_inc(
                    dma_m, 16)
                for k in range(KC):
                    sl = slice(k * P, (k + 1) * P)
                    g.dma_start(out=bq_sb[k][:, :], in_=bq_d[sl, :]).then_inc(
                        dma_m, 16)
                    g.dma_start(out=bk_sb[k][:, :], in_=bk_d[sl, :]).then_inc(
                        dma_m, 16)
                    g.dma_start(out=gsc_sb[k][:, :],
                                in_=gsc_d[sl, :]).then_inc(dma_m, 16)
                    g.dma_start(out=gbi_sb[k][:, :],
                                in_=gbi_d[sl, :]).then_inc(dma_m, 16)
                for k in range(2):
                    g.dma_start(out=x_sb[k][:, :],
                                in_=x_d[k * P:(k + 1) * P, :]).then_inc(
                        dma_x[k % 2], 16)
                # 16 weight tiles through 2 staging buffers (before x2/x3:
                # DVE converts weights first, then starts groupnorm)
                for i in range(4 * KC):
                    nm, k = W_ORDER[i // KC], i % KC
                    if i >= 2:
                        g.wait_ge(s_wcvt, 4 + i - 1)
                    g.dma_start(out=wstage[i % 2][:, :],
                                in_=w_d[nm][k * P:(k + 1) * P, :]).then_inc(
                        dma_w[i % 2], 16)
                for k in range(2, KC):
                    g.wait_ge(s_h, k - 1)       # x staging slot free
                    g.dma_start(out=x_sb[k % 2][:, :],
                                in_=x_d[k * P:(k + 1) * P, :]).then_inc(
                        dma_x[k % 2], 16)
                # output stores: 4 per quarter through 2 ot buffers
                for qq in range(NQF):
                    for o4 in range(KC):
                        n_out = 4 * qq + o4 + 1
                        g.wait_ge(s_ot, n_out)
                        g.dma_start(
                            out=out_d[o4 * P:(o4 + 1) * P,
                                      qq * F:(qq + 1) * F],
                            in_=ot[n_out % 2][:, :]).then_inc(
                            dma_o[n_out % 2], 16)

            # ================= PE: all matmuls =================
            @block.tensor
            def _(t: bass.BassEngine):
                # --- groupnorm group-combine + broadcast matmuls ---
                t.wait_ge(dma_m, 16 * NMISC)
                for k in range(KC):
                    t.wait_ge(s_dve, 21 * k + 12)       # st2 ready
                    nc.tensor.matmul(aux_ps[0:NGT, 0:2], lhsT=gmat_sb[:, :],
                                     rhs=st2[:, :], start=True,
                                     stop=True).then_inc(s_gn_pe, 1)
                    t.wait_ge(s_dve, 21 * k + 17)       # g2 (mu, rstd) ready
                    nc.tensor.matmul(aux_ps[0:P, 0:2], lhsT=gexp_sb[:, :],
                                     rhs=g2[:, :], start=True,
                                     stop=True).then_inc(s_gn_pe, 1)
                # --- qkv matmuls ---
                t.wait_ge(s_wcvt, 4 + 4 * KC)           # memsets + weights
                t.wait_ge(s_h, KC)                      # all h tiles
                for gi, grp in enumerate(qkv_groups):
                    if gi >= 2:
                        t.wait_ge(s_qg_dve, gi - 1)     # psum slot free
                    dst = s_ps[gi % 2][:, :]
                    for k in range(KC):
                        kw = dict(start=(k == 0), stop=(k == KC - 1))
                        if grp[0] == "v":
                            j = grp[1]
                            mm = nc.tensor.matmul(
                                dst, lhsT=h_sb[k][:, j * P:(j + 1) * P],
                                rhs=w_sb["wvT"][k][:, :], **kw)
                        else:
                            _, m, n = grp
                            wname = "wqT" if grp[0] == "q" else "wkT"
                            mm = nc.tensor.matmul(
                                dst,
                                lhsT=w_sb[wname][k][:, m * P:(m + 1) * P],
                                rhs=h_sb[k][:, n * F:(n + 1) * F], **kw)
                    mm.then_inc(s_qg_pe, 1)
                # --- attention + proj ---
                for qq in range(NQF):
                    qsl = slice(qq * F, (qq + 1) * F)

                    def scores(j):
                        if qq == 0 and j < 2:
                            # s_ps slots still cycling out of the qkv phase
                            t.wait_ge(s_qg_dve, NQG - 1 + j)
                        else:
                            t.wait_ge(s_exp, 32 * qq + j - 1)
                        if qq > 0 and j < 2:
                            # previous quarter's proj results still leave
                            # s_ps[j] until the ot muls read them
                            t.wait_ge(s_ot, 4 * (qq - 1) + 3 + j)
                        if j == 0 and qq > 0:
                            t.wait_ge(s_osb, 4 * qq)    # O psum slots free
                        for k in range(KC):
                            mm = nc.tensor.matmul(
                                s_ps[j % 2][:, :],
                                lhsT=k_sb[k][:, j * P:(j + 1) * P],
                                rhs=q_sb[k][:, qsl],
                                start=(k == 0), stop=(k == KC - 1))
                        mm.then_inc(s_sc, 1)

                    def attnv(j):
                        t.wait_ge(s_exp, 32 * qq + j + 1)   # probs[j] ready
                        kw = dict(start=(j == 0), stop=(j == NJ - 1))
                        nc.tensor.matmul(sums_ps[:, :], lhsT=ones_col[:, :],
                                         rhs=probs[j % 2][:, :], **kw)
                        for c4 in range(KC):
                            mm = nc.tensor.matmul(
                                o_ps[c4][:, :],
                                lhsT=vt_sb[:, j, c4 * P:(c4 + 1) * P],
                                rhs=probs[j % 2][:, :], **kw)
                        mm.then_inc(s_att, 1)

                    scores(0)
                    for j in range(1, NJ):
                        scores(j)
                        attnv(j - 1)
                    attnv(NJ - 1)
                    # broadcast 1/sums to 128 partitions (full fp32 matmul)
                    t.wait_ge(s_recip, qq + 1)
                    if qq > 0:
                        t.wait_ge(s_rb, qq)     # aux_ps read by prior rb copy
                    nc.tensor.matmul(aux_ps[:, :], lhsT=ones_row[:, :],
                                     rhs=recip[:, :], start=True,
                                     stop=True).then_inc(s_bcast, 1)
                    # proj
                    t.wait_ge(s_osb, 4 * (qq + 1))      # all o_sb drained
                    for o4 in range(KC):
                        if o4 >= 2:
                            # s_ps slot shared with proj group o4-2: wait for
                            # its ot mul to have read the result
                            t.wait_ge(s_ot, 4 * qq + o4 - 1)
                        for c4 in range(KC):
                            mm = nc.tensor.matmul(
                                s_ps[o4 % 2][:, :],
                                lhsT=w_sb["wpT"][c4][:, o4 * P:(o4 + 1) * P],
                                rhs=o_sb[c4][:, :],
                                start=(c4 == 0), stop=(c4 == KC - 1))
                        mm.then_inc(s_pp, 1)

            # ================= DVE =================
            @block.vector
            def _(v: bass.BassEngine):
                # memsets first (counted in s_wcvt), then weight converts
                nc.vector.memset(ones_col[:, :], 1.0).then_inc(s_wcvt, 1)
                nc.vector.memset(ones_row[:, :], 1.0).then_inc(s_wcvt, 1)
                nc.vector.memset(zero_col[:, :], 0.0).then_inc(s_wcvt, 1)
                nc.vector.memset(eps_sb[:, :], EPS).then_inc(s_wcvt, 1)
                for i in range(4 * KC):
                    nm, k = W_ORDER[i // KC], i % KC
                    v.wait_ge(dma_w[i % 2], 16 * (i // 2 + 1))
                    nc.vector.tensor_copy(
                        out=w_sb[nm][k][:, :],
                        in_=wstage[i % 2][:, :]).then_inc(s_wcvt, 1)
                v.wait_ge(dma_m, 16 * NMISC)
                # groupnorm: fully serialized DVE chain (s_dve), 21 ops/tile
                ndve = 0

                def step(op):
                    nonlocal ndve
                    op.then_inc(s_dve, 1)
                    ndve += 1

                for k in range(KC):
                    if k > 0:
                        v.wait_ge(s_h, k)       # previous tile fully done
                    v.wait_ge(dma_x[k % 2], 16 * (k // 2 + 1))
                    for c8 in range(HW // F):
                        if ndve:
                            v.wait_ge(s_dve, ndve)
                        step(nc.vector.bn_stats(
                            out=stats[:, c8, :],
                            in_=x_sb[k % 2][:, c8 * F:(c8 + 1) * F]))
                    v.wait_ge(s_dve, ndve)
                    step(nc.vector.bn_aggr(out=mv[:, :], in_=stats[:, :, :]))
                    v.wait_ge(s_dve, ndve)
                    step(nc.vector.tensor_copy(out=st2[:, 0:1],
                                               in_=mv[:, 0:1]))
                    v.wait_ge(s_dve, ndve)
                    step(nc.vector.tensor_mul(out=st2[:, 1:2], in0=mv[:, 0:1],
                                              in1=mv[:, 0:1]))
                    v.wait_ge(s_dve, ndve)
                    step(nc.vector.tensor_add(out=st2[:, 1:2],
                                              in0=st2[:, 1:2],
                                              in1=mv[:, 1:2]))   # 21k+12
                    v.wait_ge(s_gn_pe, 2 * k + 1)           # pg in aux_ps
                    v.wait_ge(s_dve, ndve)
                    step(nc.vector.tensor_scalar_mul(g2[:, :],
                                                     in0=aux_ps[0:NGT, 0:2],
                                                     scalar1=1.0 / GS))
                    v.wait_ge(s_dve, ndve)
                    step(nc.vector.tensor_mul(out=gv[:, :], in0=g2[:, 0:1],
                                              in1=g2[:, 0:1]))
                    v.wait_ge(s_dve, ndve)
                    step(nc.vector.tensor_sub(out=gv[:, :], in0=g2[:, 1:2],
                                              in1=gv[:, :]))     # 21k+15
                    v.wait_ge(s_gn_act, k + 1)              # sqrt done
                    step(nc.vector.reciprocal(out=gv[:, :], in_=gv[:, :]))
                    v.wait_ge(s_dve, ndve)
                    step(nc.vector.tensor_copy(out=g2[:, 1:2],
                                               in_=gv[:, :]))    # 21k+17
                    v.wait_ge(s_gn_pe, 2 * k + 2)           # pb in aux_ps
                    v.wait_ge(s_dve, ndve)
                    step(nc.vector.tensor_copy(out=chs[:, :],
                                               in_=aux_ps[0:P, 0:2]))
                    v.wait_ge(s_dve, ndve)
                    step(nc.vector.tensor_mul(out=av[:, :], in0=chs[:, 1:2],
                                              in1=gsc_sb[k][:, :]))
                    v.wait_ge(s_dve, ndve)
                    step(nc.vector.tensor_mul(out=bv_[:, :], in0=chs[:, 0:1],
                                              in1=av[:, :]))
                    v.wait_ge(s_dve, ndve)
                    step(nc.vector.tensor_sub(out=bv_[:, :],
                                              in0=gbi_sb[k][:, :],
                                              in1=bv_[:, :]))    # 21k+21
                    v.wait_ge(s_dve, ndve)
                    nc.vector.tensor_scalar(
                        out=h_sb[k][:, :], in0=x_sb[k % 2][:, :],
                        scalar1=av[:, :], scalar2=bv_[:, :],
                        op0=ALU.mult, op1=ALU.add).then_inc(s_h, 1)
                # qkv drains
                for gi, grp in enumerate(qkv_groups):
                    v.wait_ge(s_qg_pe, gi + 1)
                    src = s_ps[gi % 2][:, :]
                    if grp[0] == "v":
                        j = grp[1]
                        op = nc.vector.tensor_add(
                            out=vt_sb[:, j, :], in0=src, in1=bvb_sb[:, :])
                    elif grp[0] == "q":
                        _, m, n = grp
                        op = nc.vector.tensor_scalar_add(
                            out=q_sb[m][:, n * F:(n + 1) * F], in0=src,
                            scalar1=bq_sb[m][:, :])
                    else:
                        _, m, n = grp
                        op = nc.vector.tensor_scalar_add(
                            out=k_sb[m][:, n * F:(n + 1) * F], in0=src,
                            scalar1=bk_sb[m][:, :])
                    op.then_inc(s_qg_dve, 1)
                # attention epilogue per quarter
                for qq in range(NQF):
                    v.wait_ge(s_att, 32 * (qq + 1))
                    if qq > 0:
                        v.wait_ge(s_bcast, qq)  # recip read by prior bcast
                    nc.vector.reciprocal(
                        out=recip[:, :],
                        in_=sums_ps[:, :]).then_inc(s_recip, 1)
                    for c4 in range(KC):
                        if qq > 0:
                            v.wait_ge(s_pp, 4 * qq)     # o_sb read by proj
                        nc.vector.tensor_copy(
                            out=o_sb[c4][:, :],
                            in_=o_ps[c4][:, :]).then_inc(s_osb, 1)
                    v.wait_ge(s_bcast, qq + 1)
                    if qq > 0:
                        v.wait_ge(s_ot, 4 * qq)     # rb_sb read by prior ots
                    nc.vector.tensor_copy(
                        out=rb_sb[:, :], in_=aux_ps[:, :]).then_inc(s_rb, 1)
                    for o4 in range(KC):
                        n_out = 4 * qq + o4 + 1
                        v.wait_ge(s_pp, n_out)
                        v.wait_ge(s_rb, qq + 1)
                        if n_out > 2:
                            # store n_out-2 (same parity slot) complete
                            cnt = ((n_out - 1) // 2 if n_out % 2 == 1
                                   else (n_out - 2) // 2)
                            v.wait_ge(dma_o[n_out % 2], 16 * cnt)
                        nc.vector.tensor_mul(
                            out=ot[n_out % 2][:, :],
                            in0=s_ps[o4 % 2][:, :],
                            in1=rb_sb[:, :]).then_inc(s_ot, 1)

            # ================= ACT: sqrt + exp =================
            @block.scalar
            def _(a: bass.BassEngine):
                a.wait_ge(s_wcvt, 4)            # memsets (eps, zero) done
                for k in range(KC):
                    a.wait_ge(s_dve, 21 * k + 15)
                    nc.scalar.activation(
                        out=gv[:, :], in_=gv[:, :], func=AF.Sqrt,
                        bias=eps_sb[:, :]).then_inc(s_gn_act, 1)
                for qq in range(NQF):
                    for j in range(NJ):
                        a.wait_ge(s_sc, 32 * qq + j + 1)
                        if 32 * qq + j >= 2:
                            a.wait_ge(s_att, 32 * qq + j - 1)
                        nc.scalar.activation(
                            out=probs[j % 2][:, :], in_=s_ps[j % 2][:, :],
                            func=AF.Exp, bias=zero_col[:, :],
                            scale=SCALE).then_inc(s_exp, 1)

    return nc


def make_in_maps(x, gn_scale, gn_bias, qkv_w, qkv_b, proj_w, proj_b):
    xf = np.ascontiguousarray(x, dtype=np.float32).reshape(B, C, HW)
    wq, wk, wv = qkv_w[0:C], qkv_w[C:2 * C], qkv_w[2 * C:3 * C]
    shared = {
        "wqT": np.ascontiguousarray(wq.T, np.float32),
        "wkT": np.ascontiguousarray(wk.T, np.float32),
        "wvT": np.ascontiguousarray(wv.T, np.float32),
        "wpT": np.ascontiguousarray(proj_w.T, np.float32),
        "bq": np.ascontiguousarray(qkv_b[0:C].reshape(C, 1), np.float32),
        "bk": np.ascontiguousarray(qkv_b[C:2 * C].reshape(C, 1), np.float32),
        "bvb": np.ascontiguousarray(
            np.broadcast_to(qkv_b[2 * C:3 * C][None, :], (P, C)), np.float32),
        "gscale": np.ascontiguousarray(gn_scale.reshape(C, 1), np.float32),
        "gbias": np.ascontiguousarray(gn_bias.reshape(C, 1), np.float32),
        "gmat": np.ascontiguousarray(
            (np.arange(P)[:, None] // GS == np.arange(NGT)[None, :]),
            np.float32),
        "gexp": np.ascontiguousarray(
            (np.arange(NGT)[:, None] == np.arange(P)[None, :] // GS),
            np.float32),
    }
    in_maps = []
    for b in range(B):
        for half in range(2):
            xr = np.ascontiguousarray(np.roll(xf[b], -half * NQ, axis=1))
            in_maps.append({"x": xr, **shared})
    return in_maps, xf


def assemble(results, xf, proj_b):
    out = np.empty((B, C, HW), np.float32)
    i = 0
    for b in range(B):
        for half in range(2):
            out[b][:, half * NQ:(half + 1) * NQ] = results[i]["out"]
            i += 1
    out += np.asarray(proj_b, np.float32)[None, :, None]
    out += xf
    return out.reshape(B, C, H, W)


def kernel(x, gn_scale, gn_bias, qkv_w, qkv_b, proj_w, proj_b):
    in_maps, xf = make_in_maps(x, gn_scale, gn_bias, qkv_w, qkv_b,
                               proj_w, proj_b)
    nc = build_nc()
    res = run_bass_kernel_spmd(nc, in_maps, list(range(8)))
    return assemble(res.results, xf, proj_b)


# revision 23
# speedup vs baseline: 123.4187x; 123.4187x over previous
"""AttnBlock (GroupNorm -> 1x1 qkv conv -> full HW x HW attention -> 1x1 proj
-> residual) on 8 Trainium2 NeuronCores.

Sharding: 8 cores = 4 batch elements x 2 query-halves. Each core receives its
batch element's full x[b] (pixel axis rolled so the core's query half sits in
columns 0..2047), computes GroupNorm + full K/V + Q for its half, runs
attention over key blocks, and the output projection. The host transposes the
1x1-conv weights, adds proj bias + residual, and gathers.

Raw Bass (explicit per-engine streams + semaphores; this toolchain's walrus
rejects the multi-wait instructions Tile emits). Compute dtype bf16 for all
big matmuls (fp32 accumulation in PSUM); GroupNorm statistics in fp32.

Device layouts (partition dim first):
  h  = groupnormed x, bf16   [C=512 -> 4 tiles of 128, HW=4096]
  Q  = wqT.T @ h (+bq)       [512 -> 4 tiles, 2048]
  K  = wkT.T @ h (+bk)       [512 -> 4 tiles, 4096]
  Vt = h.T @ wvT (+bv)       [128, 32 j-blocks, 512]   (pixels on partitions)
  scores_t = K.T @ Q         [128 keys, 512 queries] psum, per (j, i-quarter)
  probs    = exp(scores * C^-0.5), bf16   (no max subtraction; |scores| < ~6)
  O       += Vt_j.T @ probs_j   [4 x [128, 512]] psum accumulated over j
  sums    += ones.T @ probs_j   [1, 512] psum
  out = (wpT.T @ O) * (1/sums broadcast)  -> DRAM [512, 2048] f32
"""

from contextlib import ExitStack

import numpy as np

import concourse.bass as bass
from concourse import mybir
from concourse.bass_utils import run_bass_kernel_spmd

F32 = mybir.dt.float32
BF16 = mybir.dt.bfloat16

B, C, H, W = 4, 512, 64, 64
HW = H * W              # 4096 pixels
NG = 32                 # groupnorm groups
GS = C // NG            # 16 channels per group
P = 128                 # SBUF partitions
KC = C // P             # 4 channel chunks
NQ = HW // 2            # 2048 queries per core
F = 512                 # free-dim tile (one PSUM bank of f32)
NJ = HW // P            # 32 key blocks
NQF = NQ // F           # 4 query quarters
NGT = P // GS           # 8 groups per channel tile
EPS = 1e-6
SCALE = float(C) ** -0.5
AF = mybir.ActivationFunctionType
ALU = mybir.AluOpType


def build_nc() -> bass.Bass:
    nc = bass.Bass()

    x_d = nc.dram_tensor("x", [C, HW], F32, kind="ExternalInput")
    w_d = {}
    for nm in ("wqT", "wkT", "wvT", "wpT"):
        w_d[nm] = nc.dram_tensor(nm, [C, C], F32, kind="ExternalInput")
    bq_d = nc.dram_tensor("bq", [C, 1], F32, kind="ExternalInput")
    bk_d = nc.dram_tensor("bk", [C, 1], F32, kind="ExternalInput")
    bvb_d = nc.dram_tensor("bvb", [P, C], F32, kind="ExternalInput")
    gsc_d = nc.dram_tensor("gscale", [C, 1], F32, kind="ExternalInput")
    gbi_d = nc.dram_tensor("gbias", [C, 1], F32, kind="ExternalInput")
    gmat_d = nc.dram_tensor("gmat", [P, NGT], F32, kind="ExternalInput")
    gexp_d = nc.dram_tensor("gexp", [NGT, P], F32, kind="ExternalInput")
    out_d = nc.dram_tensor("out", [C, NQ], F32, kind="ExternalOutput")

    ctx = ExitStack()
    with ctx:
        # ---------------- SBUF ----------------
        def sb(shape, dt, name):
            return ctx.enter_context(nc.sbuf_tensor(name, shape, dt))
        x_sb = [sb([P, HW], F32, f"x{k}") for k in range(2)]        # 32KB/p
        h_sb = [sb([P, HW], BF16, f"h{k}") for k in range(KC)]      # 32KB/p
        q_sb = [sb([P, NQ], BF16, f"q{k}") for k in range(KC)]      # 16KB/p
        k_sb = [sb([P, HW], BF16, f"kk{k}") for k in range(KC)]     # 32KB/p
        vt_sb = sb([P, NJ, F], BF16, "vt")                          # 32KB/p
        wstage = [sb([P, C], F32, f"wstage{i}") for i in range(2)]  # 4KB/p
        w_sb = {nm: [sb([P, C], BF16, f"{nm}{k}") for k in range(KC)]
                for nm in ("wqT", "wkT", "wvT", "wpT")}             # 16KB/p
        bvb_sb = sb([P, C], F32, "bvb_sb")
        gmat_sb = sb([P, NGT], F32, "gmat_sb")
        gexp_sb = sb([NGT, P], F32, "gexp_sb")
        bq_sb = [sb([P, 1], F32, f"bq_sb{k}") for k in range(KC)]
        bk_sb = [sb([P, 1], F32, f"bk_sb{k}") for k in range(KC)]
        gsc_sb = [sb([P, 1], F32, f"gsc_sb{k}") for k in range(KC)]
        gbi_sb = [sb([P, 1], F32, f"gbi_sb{k}") for k in range(KC)]
        ones_col = sb([P, 1], BF16, "ones_col")
        ones_row = sb([1, P], F32, "ones_row")
        zero_col = sb([P, 1], F32, "zero_col")
        # groupnorm scratch (per c-tile, reused)
        stats = sb([P, HW // F, 6], F32, "stats")
        mv = sb([P, 2], F32, "mv")
        st2 = sb([P, 2], F32, "st2")
        g2 = sb([NGT, 2], F32, "g2")
        gv = sb([NGT, 1], F32, "gv")
        eps_sb = sb([NGT, 1], F32, "eps_sb")
        chs = sb([P, 2], F32, "chs")
        av = sb([P, 1], F32, "av")
        bv_ = sb([P, 1], F32, "bv_")
        # attention scratch
        probs = [sb([P, F], BF16, f"probs{i}") for i in range(2)]
        recip = sb([1, F], F32, "recip")
        rb_sb = sb([P, F], F32, "rb_sb")
        o_sb = [sb([P, F], BF16, f"o_sb{i}") for i in range(KC)]
        ot = [sb([P, F], F32, f"ot{i}") for i in range(2)]

        # ---------------- PSUM (8 banks) ----------------
        def ps(shape, name):
            return ctx.enter_context(nc.psum_tensor(name, shape, F32))
        s_ps = [ps([P, F], f"s_ps{i}") for i in range(2)]
        o_ps = [ps([P, F], f"o_ps{i}") for i in range(KC)]
        sums_ps = ps([1, F], "sums_ps")
        aux_ps = ps([P, F], "aux_ps")   # gn pg/pb + recip broadcast

        # ---------------- semaphores ----------------
        def sem(name):
            return ctx.enter_context(nc.semaphore(name))
        dma_x = [sem("dma_x0"), sem("dma_x1")]   # +16 per x tile, by slot
        dma_w = [sem("dma_w0"), sem("dma_w1")]   # +16 per wstage load
        dma_m = sem("dma_m")        # +16 per misc const load
        dma_o = [sem("dma_o0"), sem("dma_o1")]   # +16 per output store
        s_wcvt = sem("s_wcvt")      # DVE memsets (4) + weight converts (16)
        s_dve = sem("s_dve")        # serialized gn DVE chain (21 per c-tile)
        s_rb = sem("s_rb")          # rb_sb copy per quarter (DVE)
        s_gn_pe = sem("s_gn_pe")    # gn PE matmuls (2 per c-tile)
        s_gn_act = sem("s_gn_act")  # gn sqrt (1 per c-tile)
        s_h = sem("s_h")            # normalized h tiles
        s_qg_pe = sem("s_qg_pe")    # qkv matmul groups done (PE)
        s_qg_dve = sem("s_qg_dve")  # qkv drains done (DVE)
        s_sc = sem("s_sc")          # scores groups (PE)
        s_exp = sem("s_exp")        # exps (ACT)
        s_att = sem("s_att")        # attnV+sums groups (PE)
        s_recip = sem("s_recip")    # recip per quarter (DVE)
        s_bcast = sem("s_bcast")    # bcast matmul per quarter (PE)
        s_osb = sem("s_osb")        # o_sb drains (DVE)
        s_pp = sem("s_pp")          # proj matmul groups (PE)
        s_ot = sem("s_ot")          # ot muls (DVE)

        NMISC = 3 + 4 * KC          # gmat, gexp, bvb, per-k consts
        W_ORDER = ("wqT", "wkT", "wvT", "wpT")

        # qkv "groups" in PE emission order
        qkv_groups = ([("v", j) for j in range(NJ)]
                      + [("q", m, n) for m in range(KC)
                         for n in range(NQ // F)]
                      + [("k", m, n) for m in range(KC)
                         for n in range(HW // F)])
        NQG = len(qkv_groups)

        with nc.Block() as block:

            # ================= GPSIMD: all DMA =================
            @block.gpsimd
            def _(g: bass.BassEngine):
                g.dma_start(out=gmat_sb[:, :], in_=gmat_d[:, :]).then_inc(
                    dma_m, 16)
                g.dma_start(out=gexp_sb[:, :], in_=gexp_d[:, :]).then_inc(
                    dma_m, 16)
                g.dma_start(out=bvb_sb[:, :], in_=bvb_d[:, :]).then_inc(
                    dma_m, 16)
                for k in range(KC):
                    sl = slice(k * P, (k + 1) * P)
                    g.dma_start(out=bq_sb[k][:, :], in_=bq_d[sl, :]).then_inc(
                        dma_m, 16)
                    g.dma_start(out=bk_sb[k][:, :], in_=bk_d[sl, :]).then_inc(
                        dma_m, 16)
                    g.dma_start(out=gsc_sb[k][:, :],
                                in_=gsc_d[sl, :]).then_inc(dma_m, 16)
                    g.dma_start(out=gbi_sb[k][:, :],
                                in_=gbi_d[sl, :]).then_inc(dma_m, 16)
                for k in range(2):
                    g.dma_start(out=x_sb[k][:, :],
                                in_=x_d[k * P:(k + 1) * P, :]).then_inc(
                        dma_x[k % 2], 16)
                # 16 weight tiles through 2 staging buffers (before x2/x3:
                # DVE converts weights first, then starts groupnorm)
                for i in range(4 * KC):
                    nm, k = W_ORDER[i // KC], i % KC
                    if i >= 2:
                        g.wait_ge(s_wcvt, 4 + i - 1)
                    g.dma_start(out=wstage[i % 2][:, :],
                                in_=w_d[nm][k * P:(k + 1) * P, :]).then_inc(
                        dma_w[i % 2], 16)
                for k in range(2, KC):
                    g.wait_ge(s_h, k - 1)       # x staging slot free
                    g.dma_start(out=x_sb[k % 2][:, :],
                                in_=x_d[k * P:(k + 1) * P, :]).then_inc(
                        dma_x[k % 2], 16)
                # output stores: 4 per quarter through 2 ot buffers
                for qq in range(NQF):
                    for o4 in range(KC):
                        n_out = 4 * qq + o4 + 1
                        g.wait_ge(s_ot, n_out)
                        g.dma_start(
                            out=out_d[o4 * P:(o4 + 1) * P,
                                      qq * F:(qq + 1) * F],
                            in_=ot[n_out % 2][:, :]).then_inc(
                            dma_o[n_out % 2], 16)

            # ================= PE: all matmuls =================
            @block.tensor
            def _(t: bass.BassEngine):
                # --- groupnorm group-combine + broadcast matmuls ---
                t.wait_ge(dma_m, 16 * NMISC)
                for k in range(KC):
                    t.wait_ge(s_dve, 21 * k + 12)       # st2 ready
                    nc.tensor.matmul(aux_ps[0:NGT, 0:2], lhsT=gmat_sb[:, :],
                                     rhs=st2[:, :], start=True,
                                     stop=True).then_inc(s_gn_pe, 1)
                    t.wait_ge(s_dve, 21 * k + 17)       # g2 (mu, rstd) ready
                    nc.tensor.matmul(aux_ps[0:P, 0:2], lhsT=gexp_sb[:, :],
                                     rhs=g2[:, :], start=True,
                                     stop=True).then_inc(s_gn_pe, 1)
                # --- qkv matmuls ---
                t.wait_ge(s_wcvt, 4 + 4 * KC)           # memsets + weights
                t.wait_ge(s_h, KC)                      # all h tiles
                for gi, grp in enumerate(qkv_groups):
                    if gi >= 2:
                        t.wait_ge(s_qg_dve, gi - 1)     # psum slot free
                    dst = s_ps[gi % 2][:, :]
                    for k in range(KC):
                        kw = dict(start=(k == 0), stop=(k == KC - 1))
                        if grp[0] == "v":
                            j = grp[1]
                            mm = nc.tensor.matmul(
                                dst, lhsT=h_sb[k][:, j * P:(j + 1) * P],
                                rhs=w_sb["wvT"][k][:, :], **kw)
                        else:
                            _, m, n = grp
                            wname = "wqT" if grp[0] == "q" else "wkT"
                            mm = nc.tensor.matmul(
                                dst,
                                lhsT=w_sb[wname][k][:, m * P:(m + 1) * P],
                                rhs=h_sb[k][:, n * F:(n + 1) * F], **kw)
                    mm.then_inc(s_qg_pe, 1)
                # --- attention + proj ---
                for qq in range(NQF):
                    qsl = slice(qq * F, (qq + 1) * F)

                    def scores(j):
                        if qq == 0 and j < 2:
                            # s_ps slots still cycling out of the qkv phase
                            t.wait_ge(s_qg_dve, NQG - 1 + j)
                        else:
                            t.wait_ge(s_exp, 32 * qq + j - 1)
                        if qq > 0 and j < 2:
                            # previous quarter's proj results still leave
                            # s_ps[j] until the ot muls read them
                            t.wait_ge(s_ot, 4 * (qq - 1) + 3 + j)
                        if j == 0 and qq > 0:
                            t.wait_ge(s_osb, 4 * qq)    # O psum slots free
                        for k in range(KC):
                            mm = nc.tensor.matmul(
                                s_ps[j % 2][:, :],
                                lhsT=k_sb[k][:, j * P:(j + 1) * P],
                                rhs=q_sb[k][:, qsl],
                                start=(k == 0), stop=(k == KC - 1))
                        mm.then_inc(s_sc, 1)

                    def attnv(j):
                        t.wait_ge(s_exp, 32 * qq + j + 1)   # probs[j] ready
                        kw = dict(start=(j == 0), stop=(j == NJ - 1))
                        nc.tensor.matmul(sums_ps[:, :], lhsT=ones_col[:, :],
                                         rhs=probs[j % 2][:, :], **kw)
                        for c4 in range(KC):
                            mm = nc.tensor.matmul(
                                o_ps[c4][:, :],
                                lhsT=vt_sb[:, j, c4 * P:(c4 + 1) * P],
                                rhs=probs[j % 2][:, :], **kw)
                        mm.then_inc(s_att, 1)

                    scores(0)
                    for j in range(1, NJ):
                        scores(j)
                        attnv(j - 1)
                    attnv(NJ - 1)
                    # broadcast 1/sums to 128 partitions (full fp32 matmul)
                    t.wait_ge(s_recip, qq + 1)
                    if qq > 0:
                        t.wait_ge(s_rb, qq)     # aux_ps read by prior rb copy
                    nc.tensor.matmul(aux_ps[:, :], lhsT=ones_row[:, :],
                                     rhs=recip[:, :], start=True,
                                     stop=True).then_inc(s_bcast, 1)
                    # proj
                    t.wait_ge(s_osb, 4 * (qq + 1))      # all o_sb drained
                    for o4 in range(KC):
                        if o4 >= 2:
                            # s_ps slot shared with proj group o4-2: wait for
                            # its ot mul to have read the result
                            t.wait_ge(s_ot, 4 * qq + o4 - 1)
                        for c4 in range(KC):
                            mm = nc.tensor.matmul(
                                s_ps[o4 % 2][:, :],
                                lhsT=w_sb["wpT"][c4][:, o4 * P:(o4 + 1) * P],
                                rhs=o_sb[c4][:, :],
                                start=(c4 == 0), stop=(c4 == KC - 1))
                        mm.then_inc(s_pp, 1)

            # ================= DVE =================
            @block.vector
            def _(v: bass.BassEngine):
                # memsets first (counted in s_wcvt), then weight converts
                nc.vector.memset(ones_col[:, :], 1.0).then_inc(s_wcvt, 1)
                nc.vector.memset(ones_row[:, :], 1.0).then_inc(s_wcvt, 1)
                nc.vector.memset(zero_col[:, :], 0.0).then_inc(s_wcvt, 1)
                nc.vector.memset(eps_sb[:, :], EPS).then_inc(s_wcvt, 1)
                for i in range(4 * KC):
                    nm, k = W_ORDER[i // KC], i % KC
                    v.wait_ge(dma_w[i % 2], 16 * (i // 2 + 1))
                    nc.vector.tensor_copy(
                        out=w_sb[nm][k][:, :],
                        in_=wstage[i % 2][:, :]).then_inc(s_wcvt, 1)
                v.wait_ge(dma_m, 16 * NMISC)
                # groupnorm: fully serialized DVE chain (s_dve), 21 ops/tile
                ndve = 0

                def step(op):
                    nonlocal ndve
                    op.then_inc(s_dve, 1)
                    ndve += 1

                for k in range(KC):
                    if k > 0:
                        v.wait_ge(s_h, k)       # previous tile fully done
                    v.wait_ge(dma_x[k % 2], 16 * (k // 2 + 1))
                    for c8 in range(HW // F):
                        if ndve:
                            v.wait_ge(s_dve, ndve)
                        step(nc.vector.bn_stats(
                            out=stats[:, c8, :],
                            in_=x_sb[k % 2][:, c8 * F:(c8 + 1) * F]))
                    v.wait_ge(s_dve, ndve)
                    step(nc.vector.bn_aggr(out=mv[:, :], in_=stats[:, :, :]))
                    v.wait_ge(s_dve, ndve)
                    step(nc.vector.tensor_copy(out=st2[:, 0:1],
                                               in_=mv[:, 0:1]))
                    v.wait_ge(s_dve, ndve)
                    step(nc.vector.tensor_mul(out=st2[:, 1:2], in0=mv[:, 0:1],
                                              in1=mv[:, 0:1]))
                    v.wait_ge(s_dve, ndve)
                    step(nc.vector.tensor_add(out=st2[:, 1:2],
                                              in0=st2[:, 1:2],
                                              in1=mv[:, 1:2]))   # 21k+12
                    v.wait_ge(s_gn_pe, 2 * k + 1)           # pg in aux_ps
                    v.wait_ge(s_dve, ndve)
                    step(nc.vector.tensor_scalar_mul(g2[:, :],
                                                     in0=aux_ps[0:NGT, 0:2],
                                                     scalar1=1.0 / GS))
                    v.wait_ge(s_dve, ndve)
                    step(nc.vector.tensor_mul(out=gv[:, :], in0=g2[:, 0:1],
                                              in1=g2[:, 0:1]))
                    v.wait_ge(s_dve, ndve)
                    step(nc.vector.tensor_sub(out=gv[:, :], in0=g2[:, 1:2],
                                              in1=gv[:, :]))     # 21k+15
                    v.wait_ge(s_gn_act, k + 1)              # sqrt done
                    step(nc.vector.reciprocal(out=gv[:, :], in_=gv[:, :]))
                    v.wait_ge(s_dve, ndve)
                    step(nc.vector.tensor_copy(out=g2[:, 1:2],
                                               in_=gv[:, :]))    # 21k+17
                    v.wait_ge(s_gn_pe, 2 * k + 2)           # pb in aux_ps
                    v.wait_ge(s_dve, ndve)
                    step(nc.vector.tensor_copy(out=chs[:, :],
                                               in_=aux_ps[0:P, 0:2]))
                    v.wait_ge(s_dve, ndve)
                    step(nc.vector.tensor_mul(out=av[:, :], in0=chs[:, 1:2],
                                              in1=gsc_sb[k][:, :]))
                    v.wait_ge(s_dve, ndve)
                    step(nc.vector.tensor_mul(out=bv_[:, :], in0=chs[:, 0:1],
                                              in1=av[:, :]))
                    v.wait_ge(s_dve, ndve)
                    step(nc.vector.tensor_sub(out=bv_[:, :],
                                              in0=gbi_sb[k][:, :],
                                              in1=bv_[:, :]))    # 21k+21
                    v.wait_ge(s_dve, ndve)
                    nc.vector.tensor_scalar(
                        out=h_sb[k][:, :], in0=x_sb[k % 2][:, :],
                        scalar1=av[:, :], scalar2=bv_[:, :],
                        op0=ALU.mult, op1=ALU.add).then_inc(s_h, 1)
                # qkv drains
                for gi, grp in enumerate(qkv_groups):
                    v.wait_ge(s_qg_pe, gi + 1)
                    src = s_ps[gi % 2][:, :]
                    if grp[0] == "v":
                        j = grp[1]
                        op = nc.vector.tensor_add(
                            out=vt_sb[:, j, :], in0=src, in1=bvb_sb[:, :])
                    elif grp[0] == "q":
                        _, m, n = grp
                        op = nc.vector.tensor_scalar_add(
                            out=q_sb[m][:, n * F:(n + 1) * F], in0=src,
                            scalar1=bq_sb[m][:, :])
                    else:
                        _, m, n = grp
                        op = nc.vector.tensor_scalar_add(
                            out=k_sb[m][:, n * F:(n + 1) * F], in0=src,
                            scalar1=bk_sb[m][:, :])
                    op.then_inc(s_qg_dve, 1)
                # attention epilogue per quarter
                for qq in range(NQF):
                    v.wait_ge(s_att, 32 * (qq + 1))
                    if qq > 0:
                        v.wait_ge(s_bcast, qq)  # recip read by prior bcast
                    nc.vector.reciprocal(
                        out=recip[:, :],
                        in_=sums_ps[:, :]).then_inc(s_recip, 1)
                    for c4 in range(KC):
                        if qq > 0:
                            v.wait_ge(s_pp, 4 * qq)     # o_sb read by proj
                        nc.vector.tensor_copy(
                            out=o_sb[c4][:, :],
                            in_=o_ps[c4][:, :]).then_inc(s_osb, 1)
                    v.wait_ge(s_bcast, qq + 1)
                    if qq > 0:
                        v.wait_ge(s_ot, 4 * qq)     # rb_sb read by prior ots
                    nc.vector.tensor_copy(
                        out=rb_sb[:, :], in_=aux_ps[:, :]).then_inc(s_rb, 1)
                    for o4 in range(KC):
                        n_out = 4 * qq + o4 + 1
                        v.wait_ge(s_pp, n_out)
                        v.wait_ge(s_rb, qq + 1)
                        if n_out > 2:
                            # store n_out-2 (same parity slot) complete
                            cnt = ((n_out - 1) // 2 if n_out % 2 == 1
                                   else (n_out - 2) // 2)
                            v.wait_ge(dma_o[n_out % 2], 16 * cnt)
                        nc.vector.tensor_mul(
                            out=ot[n_out % 2][:, :],
                            in0=s_ps[o4 % 2][:, :],
                            in1=rb_sb[:, :]).then_inc(s_ot, 1)

            # ================= ACT: sqrt + exp =================
            @block.scalar
            def _(a: bass.BassEngine):
                a.wait_ge(s_wcvt, 4)            # memsets (eps, zero) done
                for k in range(KC):
                    a.wait_ge(s_dve, 21 * k + 15)
                    nc.scalar.activation(
                        out=gv[:, :], in_=gv[:, :], func=AF.Sqrt,
                        bias=eps_sb[:, :]).then_inc(s_gn_act, 1)
                for qq in range(NQF):
                    for j in range(NJ):
                        a.wait_ge(s_sc, 32 * qq + j + 1)
                        if 32 * qq + j >= 2:
                            a.wait_ge(s_att, 32 * qq + j - 1)
                        nc.scalar.activation(
                            out=probs[j % 2][:, :], in_=s_ps[j % 2][:, :],
                            func=AF.Exp, bias=zero_col[:, :],
                            scale=SCALE).then_inc(s_exp, 1)

    return nc


def make_in_maps(x, gn_scale, gn_bias, qkv_w, qkv_b, proj_w, proj_b):
    xf = np.ascontiguousarray(x, dtype=np.float32).reshape(B, C, HW)
    wq, wk, wv = qkv_w[0:C], qkv_w[C:2 * C], qkv_w[2 * C:3 * C]
    shared = {
        "wqT": np.ascontiguousarray(wq.T, np.float32),
        "wkT": np.ascontiguousarray(wk.T, np.float32),
        "wvT": np.ascontiguousarray(wv.T, np.float32),
        "wpT": np.ascontiguousarray(proj_w.T, np.float32),
        "bq": np.ascontiguousarray(qkv_b[0:C].reshape(C, 1), np.float32),
        "bk": np.ascontiguousarray(qkv_b[C:2 * C].reshape(C, 1), np.float32),
        "bvb": np.ascontiguousarray(
            np.broadcast_to(qkv_b[2 * C:3 * C][None, :], (P, C)), np.float32),
        "gscale": np.ascontiguousarray(gn_scale.reshape(C, 1), np.float32),
        "gbias": np.ascontiguousarray(gn_bias.reshape(C, 1), np.float32),
        "gmat": np.ascontiguousarray(
            (np.arange(P)[:, None] // GS == np.arange(NGT)[None, :]),
            np.float32),
        "gexp": np.ascontiguousarray(
            (np.arange(NGT)[:, None] == np.arange(P)[None, :] // GS),
            np.float32),
    }
    in_maps = []
    for b in range(B):
        for half in range(2):
            xr = np.ascontiguousarray(np.roll(xf[b], -half * NQ, axis=1))
            in_maps.append({"x": xr, **shared})
    return in_maps, xf


def assemble(results, xf, proj_b):
    out = np.empty((B, C, HW), np.float32)
    i = 0
    for b in range(B):
        for half in range(2):
            out[b][:, half * NQ:(half + 1) * NQ] = results[i]["out"]
            i += 1
    out += np.asarray(proj_b, np.float32)[None, :, None]
    out += xf
    return out.reshape(B, C, H, W)


def kernel(x, gn_scale, gn_bias, qkv_w, qkv_b, proj_w, proj_b):
    in_maps, xf = make_in_maps(x, gn_scale, gn_bias, qkv_w, qkv_b,
                               proj_w, proj_b)
    nc = build_nc()
    res = run_bass_kernel_spmd(nc, in_maps, list(range(8)))
    return assemble(res.results, xf, proj_b)
